# revision 1
# baseline (speedup 1.0000x reference)
"""MemoryReader attention kernel for 8x Trainium2 NeuronCores.

Computation per batch (B=16, CK=64, CV=512, N=HW=3136):
    scores[n, m] = (2 * mk_f[:,n]@qk_f[:,m] - ||mk_f[:,n]||^2) / sqrt(CK)
    A = softmax(scores, axis=n)
    mem[c, m]  = sum_n mv_f[c, n] * A[n, m]
    outputs: (mem, qv)   -- qv is a pure passthrough (host-side).

Sharding: data-parallel over batch, 2 batches per core.

Per-core layout ("option A"): memory dim n on PSUM partitions (25 tiles of
128), query dim m on the free axis (8 chunks of 392).  The -||mk||^2 bias is
folded into the score matmul as an augmented contraction row (K=65).  exp runs
on ScalarE straight out of PSUM with scale=1/4.  Readout accumulates over the
25 n-tiles with transposed mv as stationary weights; the softmax denominator
comes from a ones-vector matmul accumulated alongside, its reciprocal via
exp(-ln(x)) (same ACT table set as exp), broadcast across partitions with a
K=1 matmul, applied by VectorE after the PSUM has been evacuated.

All matmuls run in float32r (rounded fp32, full PE rate at N>=256; every
operand is produced by a DVE/ACT op that performs the rounding).
"""
import sys

if "/opt/trn_rl_repo" not in sys.path:
    sys.path.insert(0, "/opt/trn_rl_repo")

import numpy as np

import concourse.bacc as bacc
import concourse.mybir as mybir
import concourse.tile as tile
from concourse.bass_utils import run_bass_kernel_spmd

F32 = mybir.dt.float32
F32R = mybir.dt.float32r
EXP = mybir.ActivationFunctionType.Exp
LN = mybir.ActivationFunctionType.Ln

B, CK, CV, H, W = 16, 64, 512, 56, 56
N = H * W                      # 3136: memory positions == query positions
NB = 2                         # batches per core
NCORES = 8
NT = 25                        # n tiles: 24*128 + 64
CH = 8                         # m chunks
MC = N // CH                   # 392
CT = CV // 128                 # 4 c tiles


def _ntile(t):
    return (t * 128, 64 if t == NT - 1 else 128)


def build_nc():
    nc = bacc.Bacc("TRN2", target_bir_lowering=False, debug=False,
                   num_devices=NCORES)
    mk_d = nc.dram_tensor("mk", [NB, CK, N], F32, kind="ExternalInput")
    qk_d = nc.dram_tensor("qk", [NB, CK, N], F32, kind="ExternalInput")
    mv_d = nc.dram_tensor("mv", [NB, CV, N], F32, kind="ExternalInput")
    id_d = nc.dram_tensor("ident", [128, 128], F32, kind="ExternalInput")
    out_d = nc.dram_tensor("out", [NB, CV, N], F32, kind="ExternalOutput")

    with tile.TileContext(nc) as tc:
        with (
            tc.tile_pool(name="const", bufs=1) as constp,
            tc.tile_pool(name="stage", bufs=3) as stagep,
            tc.tile_pool(name="msq", bufs=2) as msqp,
            tc.tile_pool(name="mvchunk", bufs=2) as mvchunkp,
            tc.tile_pool(name="mvt", bufs=1) as mvtp,
            tc.tile_pool(name="aug", bufs=1) as augp,
            tc.tile_pool(name="ebuf", bufs=2) as ep,
            tc.tile_pool(name="osb", bufs=2) as outp,
            tc.tile_pool(name="bcsb", bufs=2) as bcp,
            tc.tile_pool(name="row", bufs=2) as rowp,
            tc.tile_pool(name="ps_s", bufs=1, space="PSUM") as ps_s,
            tc.tile_pool(name="ps_mem", bufs=1, space="PSUM") as ps_mem,
            tc.tile_pool(name="ps_cs", bufs=1, space="PSUM") as ps_cs,
            tc.tile_pool(name="ps_tr", bufs=2, space="PSUM") as ps_tr,
        ):
            # constants
            ident = constp.tile([128, 128], F32)
            nc.sync.dma_start(ident[:], id_d[:])
            onesf = constp.tile([128, 1], F32)
            nc.gpsimd.memset(onesf[:], 1.0)
            ones128 = constp.tile([128, 1], F32R)
            nc.vector.tensor_copy(ones128[:], onesf[:])
            onesbf = constp.tile([1, 128], F32)
            nc.gpsimd.memset(onesbf[:], 1.0)
            onesb = constp.tile([1, 128], F32R)
            nc.vector.tensor_copy(onesb[:], onesbf[:])
            negh_f = constp.tile([1, MC], F32)
            nc.gpsimd.memset(negh_f[:], -0.5)

            pending = [None]  # deferred per-chunk tail emission

            def chunk_tail(u, cs_ps, mem_ps, bc_sb, out_sb, mvt, E, b):
                # reciprocal of the softmax denominator: exp(-ln(x))
                lnr = rowp.tile([1, MC], F32, tag="ln")
                nc.scalar.activation(lnr[:], cs_ps[0:1, 0:MC], LN)
                rcr = rowp.tile([1, MC], F32R, tag="rc")
                nc.scalar.activation(rcr[:], lnr[:], EXP, scale=-1.0)

                def emit_bc():
                    # broadcast recip across 128 partitions via K=1 matmul
                    bc_ps = ps_tr.tile([128, 512], F32, tag="tr")
                    nc.tensor.matmul(bc_ps[:, 0:MC], onesb[:], rcr[:],
                                     start=True, stop=True)
                    nc.scalar.copy(bc_sb[:], bc_ps[:, 0:MC])
                    for c in range(CT):
                        nc.vector.tensor_mul(out_sb[:, c, :], out_sb[:, c, :],
                                             bc_sb[:])
                    dst = out_d[b].rearrange("(ct p) n -> p ct n", p=128)
                    nc.sync.dma_start(dst[:, :, u * MC:(u + 1) * MC], out_sb[:])
                return emit_bc

            for b in range(NB):
                # ---- startup: build mk_aug (rows 0-63 = mk, row 64 = ||mk||^2),
                #      qk_aug (rows 0-63 = qk, row 64 = -0.5), mvT ----
                mk_aug = augp.tile([65, N], F32R, tag="mkaug")
                qk_aug = augp.tile([65, N], F32R, tag="qkaug")
                for u in range(CH):
                    sl = slice(u * MC, (u + 1) * MC)
                    st = stagep.tile([CK, MC], F32, tag="stage")
                    nc.sync.dma_start(st[:], mk_d[b, :, sl])
                    nc.vector.tensor_copy(mk_aug[0:CK, sl], st[:])
                    msq = msqp.tile([CK, MC], F32R, tag="msq")
                    nc.vector.tensor_mul(msq[:], st[:], st[:])
                    a_ps = ps_cs.tile([1, 512], F32, tag="cs")
                    nc.tensor.matmul(a_ps[:, 0:MC], ones128[0:CK, :], msq[:],
                                     start=True, stop=True)
                    nc.scalar.copy(mk_aug[CK:CK + 1, sl], a_ps[0:1, 0:MC])
                    st2 = stagep.tile([CK, MC], F32, tag="stage")
                    nc.sync.dma_start(st2[:], qk_d[b, :, sl])
                    nc.vector.tensor_copy(qk_aug[0:CK, sl], st2[:])
                    nc.vector.tensor_copy(qk_aug[CK:CK + 1, sl], negh_f[:])

                mvt = mvtp.tile([128, NT, 512], F32R, tag="mvt")
                mv_r = mv_d[b].rearrange("(ct p) n -> p ct n", p=128)
                for t in range(NT):
                    n0, pt = _ntile(t)
                    mvc = mvchunkp.tile([128, CT, 128], F32, tag="mvchunk")
                    nc.sync.dma_start(mvc[:, :, 0:pt], mv_r[:, :, n0:n0 + pt])
                    tr = ps_tr.tile([128, 512], F32, tag="tr")
                    for c in range(CT):
                        nc.tensor.transpose(tr[0:pt, c * 128:(c + 1) * 128],
                                            mvc[:, c, 0:pt], ident[:])
                    nc.vector.tensor_copy(mvt[0:pt, t, :], tr[0:pt, :])

                # ---- main: per m-chunk flash pipeline ----
                for u in range(CH):
                    sl = slice(u * MC, (u + 1) * MC)
                    E = ep.tile([128, NT, MC], F32R, tag="E")
                    mem_ps = ps_mem.tile([128, CT, 512], F32, tag="mem")
                    cs_ps = ps_cs.tile([1, 512], F32, tag="cs")
                    out_sb = outp.tile([128, CT, MC], F32, tag="osb")
                    bc_sb = bcp.tile([128, MC], F32, tag="bc")

                    s_ps = [None, None]

                    def mm1(t):
                        n0, pt = _ntile(t)
                        sp = ps_s.tile([128, 512], F32, tag="s")
                        nc.tensor.matmul(sp[0:pt, 0:MC],
                                         mk_aug[:, n0:n0 + pt],
                                         qk_aug[:, sl],
                                         start=True, stop=True)
                        nc.scalar.activation(E[0:pt, t, :], sp[0:pt, 0:MC],
                                             EXP, scale=0.25)

                    def mm2grp(t):
                        n0, pt = _ntile(t)
                        first, last = t == 0, t == NT - 1
                        nc.tensor.matmul(cs_ps[:, 0:MC], ones128[0:pt, :],
                                         E[0:pt, t, :], start=first, stop=last)
                        for c in range(CT):
                            nc.tensor.matmul(mem_ps[:, c, 0:MC],
                                             mvt[0:pt, t, c * 128:(c + 1) * 128],
                                             E[0:pt, t, :],
                                             start=first, stop=last)
                            if last:
                                nc.scalar.copy(out_sb[:, c, :],
                                               mem_ps[:, c, 0:MC])

                    mm1(0)
                    mm1(1)
                    if pending[0] is not None:
                        pending[0]()
                    for t in range(2, NT):
                        mm1(t)
                        mm2grp(t - 2)
                    mm2grp(NT - 2)
                    mm2grp(NT - 1)
                    pending[0] = chunk_tail(u, cs_ps, mem_ps, bc_sb, out_sb,
                                            mvt, E, b)
            pending[0]()

    nc.compile()
    return nc


_NC = None


def _get_nc():
    global _NC
    if _NC is None:
        _NC = build_nc()
    return _NC


def kernel(mk, qk, mv, qv):
    mk = np.ascontiguousarray(np.asarray(mk, dtype=np.float32)).reshape(B, CK, N)
    qk = np.ascontiguousarray(np.asarray(qk, dtype=np.float32)).reshape(B, CK, N)
    mv = np.ascontiguousarray(np.asarray(mv, dtype=np.float32)).reshape(B, CV, N)
    ident = np.eye(128, dtype=np.float32)

    nc = _get_nc()
    in_maps = [
        {"mk": mk[NB * i:NB * (i + 1)],
         "qk": qk[NB * i:NB * (i + 1)],
         "mv": mv[NB * i:NB * (i + 1)],
         "ident": ident}
        for i in range(NCORES)
    ]
    res = run_bass_kernel_spmd(nc, in_maps, list(range(NCORES)))
    mem = np.concatenate([res.results[i]["out"] for i in range(NCORES)], axis=0)
    mem = mem.reshape(B, CV, H, W)
    return mem, np.asarray(qv)


# revision 6
# speedup vs baseline: 153.0236x; 153.0236x over previous
"""MemoryReader attention kernel for 8x Trainium2 NeuronCores.

Computation per batch (B=16, CK=64, CV=512, N=HW=3136):
    scores[n, m] = (2 * mk_f[:,n]@qk_f[:,m] - ||mk_f[:,n]||^2) / sqrt(CK)
    A = softmax(scores, axis=n)
    mem[c, m]  = sum_n mv_f[c, n] * A[n, m]
    outputs: (mem, qv)   -- qv is a pure passthrough (host-side).

Sharding: data-parallel over batch, 2 batches per core.

Per-core layout ("option A"): memory dim n on PSUM partitions (25 tiles of
128), query dim m on the free axis (8 chunks of 392).  The -||mk||^2 bias is
folded into the score matmul as an augmented contraction row (K=65).  exp runs
on ScalarE straight out of PSUM with scale=1/4.  Readout accumulates over the
25 n-tiles with transposed mv as stationary weights; the softmax denominator
comes from a ones-vector matmul accumulated alongside, its reciprocal via
exp(-ln(x)) (same ACT table set as exp), broadcast across partitions with a
K=1 matmul, applied by VectorE after the PSUM has been evacuated.

All matmuls run in float32r (rounded fp32, full PE rate at N>=256; every
operand is produced by a DVE/ACT op that performs the rounding).
"""
import sys

if "/opt/trn_rl_repo" not in sys.path:
    sys.path.insert(0, "/opt/trn_rl_repo")

import numpy as np

import concourse.bacc as bacc
import concourse.mybir as mybir
import concourse.tile as tile
from concourse.bass_utils import run_bass_kernel_spmd

F32 = mybir.dt.float32
F32R = mybir.dt.float32r
EXP = mybir.ActivationFunctionType.Exp
LN = mybir.ActivationFunctionType.Ln

B, CK, CV, H, W = 16, 64, 512, 56, 56
N = H * W                      # 3136: memory positions == query positions
NB = 2                         # batches per core
NCORES = 8
NT = 25                        # n tiles: 24*128 + 64
CH = 8                         # m chunks
MC = N // CH                   # 392
CT = CV // 128                 # 4 c tiles


def _ntile(t):
    return (t * 128, 64 if t == NT - 1 else 128)


def build_nc(rep=1, loop=0):
    nc = bacc.Bacc("TRN2", target_bir_lowering=False, debug=False,
                   num_devices=NCORES)
    mk_d = nc.dram_tensor("mk", [NB, CK, N], F32, kind="ExternalInput")
    qk_d = nc.dram_tensor("qk", [NB, CK, N], F32, kind="ExternalInput")
    mv_d = nc.dram_tensor("mv", [NB, CV, N], F32, kind="ExternalInput")
    id_d = nc.dram_tensor("ident", [128, 128], F32, kind="ExternalInput")
    out_d = nc.dram_tensor("out", [NB, CV, N], F32, kind="ExternalOutput")

    with tile.TileContext(nc) as tc:
        with (
            tc.tile_pool(name="const", bufs=1) as constp,
            tc.tile_pool(name="stage", bufs=3) as stagep,
            tc.tile_pool(name="msq", bufs=2) as msqp,
            tc.tile_pool(name="mvchunk", bufs=2) as mvchunkp,
            tc.tile_pool(name="mvt", bufs=1) as mvtp,
            tc.tile_pool(name="aug", bufs=1) as augp,
            tc.tile_pool(name="ebuf", bufs=2) as ep,
            tc.tile_pool(name="osb", bufs=2) as outp,
            tc.tile_pool(name="bcsb", bufs=2) as bcp,
            tc.tile_pool(name="row", bufs=2) as rowp,
            tc.tile_pool(name="ps_s", bufs=1, space="PSUM") as ps_s,
            tc.tile_pool(name="ps_mem", bufs=1, space="PSUM") as ps_mem,
            tc.tile_pool(name="ps_cs", bufs=1, space="PSUM") as ps_cs,
            tc.tile_pool(name="ps_tr", bufs=2, space="PSUM") as ps_tr,
        ):
            # constants
            ident = constp.tile([128, 128], F32)
            nc.sync.dma_start(ident[:], id_d[:])
            onesf = constp.tile([128, 1], F32)
            nc.gpsimd.memset(onesf[:], 1.0)
            ones128 = constp.tile([128, 1], F32R)
            nc.vector.tensor_copy(ones128[:], onesf[:])
            onesbf = constp.tile([1, 128], F32)
            nc.gpsimd.memset(onesbf[:], 1.0)
            onesb = constp.tile([1, 128], F32R)
            nc.vector.tensor_copy(onesb[:], onesbf[:])
            negh_f = constp.tile([1, MC], F32)
            nc.gpsimd.memset(negh_f[:], -0.5)

            pending = [None]  # deferred per-chunk tail emission

            def chunk_tail(u, cs_ps, mem_ps, bc_sb, out_sb, mvt, E, b):
                # reciprocal of the softmax denominator: exp(-ln(x))
                lnr = rowp.tile([1, MC], F32, tag="ln")
                nc.scalar.activation(lnr[:], cs_ps[0:1, 0:MC], LN)
                rcr = rowp.tile([1, MC], F32R, tag="rc")
                nc.scalar.activation(rcr[:], lnr[:], EXP, scale=-1.0)

                def emit_bc():
                    # broadcast recip across 128 partitions via K=1 matmul
                    bc_ps = ps_tr.tile([128, 512], F32, tag="tr")
                    nc.tensor.matmul(bc_ps[:, 0:MC], onesb[:], rcr[:],
                                     start=True, stop=True)
                    nc.scalar.copy(bc_sb[:], bc_ps[:, 0:MC])
                    for c in range(CT):
                        nc.vector.tensor_mul(out_sb[:, c, :], out_sb[:, c, :],
                                             bc_sb[:])
                    dst = out_d[b].rearrange("(ct p) n -> p ct n", p=128)
                    nc.sync.dma_start(dst[:, :, u * MC:(u + 1) * MC], out_sb[:])
                return emit_bc

            def body():
              for b in [b for _ in range(rep) for b in range(NB)]:
                # ---- startup: build mk_aug (rows 0-63 = mk, row 64 = ||mk||^2),
                #      qk_aug (rows 0-63 = qk, row 64 = -0.5), mvT ----
                mk_aug = augp.tile([65, N], F32R, tag="mkaug")
                qk_aug = augp.tile([65, N], F32R, tag="qkaug")
                for u in range(CH):
                    sl = slice(u * MC, (u + 1) * MC)
                    st = stagep.tile([CK, MC], F32, tag="stage")
                    nc.sync.dma_start(st[:], mk_d[b, :, sl])
                    nc.vector.tensor_copy(mk_aug[0:CK, sl], st[:])
                    msq = msqp.tile([CK, MC], F32R, tag="msq")
                    nc.vector.tensor_mul(msq[:], st[:], st[:])
                    a_ps = ps_cs.tile([1, 512], F32, tag="cs")
                    nc.tensor.matmul(a_ps[:, 0:MC], ones128[0:CK, :], msq[:],
                                     start=True, stop=True)
                    nc.scalar.copy(mk_aug[CK:CK + 1, sl], a_ps[0:1, 0:MC])
                    st2 = stagep.tile([CK, MC], F32, tag="stage")
                    nc.sync.dma_start(st2[:], qk_d[b, :, sl])
                    nc.vector.tensor_copy(qk_aug[0:CK, sl], st2[:])
                    nc.vector.tensor_copy(qk_aug[CK:CK + 1, sl], negh_f[:])

                mvt = mvtp.tile([128, NT, 512], F32R, tag="mvt")
                mv_r = mv_d[b].rearrange("(ct p) n -> p ct n", p=128)
                for t in range(NT):
                    n0, pt = _ntile(t)
                    mvc = mvchunkp.tile([128, CT, 128], F32, tag="mvchunk")
                    nc.sync.dma_start(mvc[:, :, 0:pt], mv_r[:, :, n0:n0 + pt])
                    tr = ps_tr.tile([128, 512], F32, tag="tr")
                    for c in range(CT):
                        nc.tensor.transpose(tr[0:pt, c * 128:(c + 1) * 128],
                                            mvc[:, c, 0:pt], ident[:])
                    nc.vector.tensor_copy(mvt[0:pt, t, :], tr[0:pt, :])

                # ---- main: per m-chunk flash pipeline ----
                for u in range(CH):
                    sl = slice(u * MC, (u + 1) * MC)
                    E = ep.tile([128, NT, MC], F32R, tag="E")
                    mem_ps = ps_mem.tile([128, CT, 512], F32, tag="mem")
                    cs_ps = ps_cs.tile([1, 512], F32, tag="cs")
                    out_sb = outp.tile([128, CT, MC], F32, tag="osb")
                    bc_sb = bcp.tile([128, MC], F32, tag="bc")

                    s_ps = [None, None]

                    def mm1(t):
                        n0, pt = _ntile(t)
                        sp = ps_s.tile([128, 512], F32, tag="s")
                        nc.tensor.matmul(sp[0:pt, 0:MC],
                                         mk_aug[:, n0:n0 + pt],
                                         qk_aug[:, sl],
                                         start=True, stop=True)
                        nc.scalar.activation(E[0:pt, t, :], sp[0:pt, 0:MC],
                                             EXP, scale=0.25)

                    def mm2grp(t):
                        n0, pt = _ntile(t)
                        first, last = t == 0, t == NT - 1
                        nc.tensor.matmul(cs_ps[:, 0:MC], ones128[0:pt, :],
                                         E[0:pt, t, :], start=first, stop=last)
                        for c in range(CT):
                            nc.tensor.matmul(mem_ps[:, c, 0:MC],
                                             mvt[0:pt, t, c * 128:(c + 1) * 128],
                                             E[0:pt, t, :],
                                             start=first, stop=last)
                            if last:
                                nc.scalar.copy(out_sb[:, c, :],
                                               mem_ps[:, c, 0:MC])

                    mm1(0)
                    mm1(1)
                    if pending[0] is not None:
                        pending[0]()
                    for t in range(2, NT):
                        mm1(t)
                        mm2grp(t - 2)
                    mm2grp(NT - 2)
                    mm2grp(NT - 1)
                    pending[0] = chunk_tail(u, cs_ps, mem_ps, bc_sb, out_sb,
                                            mvt, E, b)
              pending[0]()
              pending[0] = None

            if loop:
                with tc.For_i(0, loop, 1):
                    body()
            else:
                body()

    nc.compile()
    return nc


_NC = None


def _get_nc():
    global _NC
    if _NC is None:
        _NC = build_nc()
    return _NC


def kernel(mk, qk, mv, qv):
    mk = np.ascontiguousarray(np.asarray(mk, dtype=np.float32)).reshape(B, CK, N)
    qk = np.ascontiguousarray(np.asarray(qk, dtype=np.float32)).reshape(B, CK, N)
    mv = np.ascontiguousarray(np.asarray(mv, dtype=np.float32)).reshape(B, CV, N)
    ident = np.eye(128, dtype=np.float32)

    nc = _get_nc()
    in_maps = [
        {"mk": mk[NB * i:NB * (i + 1)],
         "qk": qk[NB * i:NB * (i + 1)],
         "mv": mv[NB * i:NB * (i + 1)],
         "ident": ident}
        for i in range(NCORES)
    ]
    res = run_bass_kernel_spmd(nc, in_maps, list(range(NCORES)))
    mem = np.concatenate([res.results[i]["out"] for i in range(NCORES)], axis=0)
    mem = mem.reshape(B, CV, H, W)
    return mem, np.asarray(qv)
